# revision 1
# baseline (speedup 1.0000x reference)
"""Decorrelation (ZCA-whitening) normalization kernel for Trainium2 (Bass/Tile).

Full input (64, 56, 56, 256) f32. Data-parallel over batch across 8 NeuronCores
(8 batches -> 25088 pixels per core). Per core:

  Pass 1: stream pixel-major (128px, 14, 256ch) f32 chunks from HBM, cast to
          fp16, accumulate per-half 128x128 second-moment Gram blocks on the
          PE (PSUM f32), PE-transpose every (128px,128ch) tile to channel-major
          fp16 and keep it fully SBUF-resident (12.8 MB), reduce channel sums.
  Stats:  one 132KB AllReduce of [G_a | G_b | sum_a | sum_b] across the 8
          cores; each core then runs the tiny (2 x 128x128 block-diagonal)
          Newton-Schulz iteration in f32 on-device, producing the whitening
          matrix wm (fp16) and the -mean bias.
  Pass 2: subtract mean in-place on the channel-major resident tiles (ACT,
          per-partition bias), whitening matmul lhsT=resident_slice rhs=wm
          (fp16 -> PSUM f32, output pixel-major), copy to staging, DMA out.

HBM traffic per core = 1x read + 1x write (pass 2 reads nothing from HBM).
"""

import sys

import numpy as np

for _p in ("/root/.axon_site/_ro/trn_rl_repo", "/opt/trn_rl_repo"):
    if _p not in sys.path:
        sys.path.append(_p)

# ---------------------------------------------------------------- constants
B, W, H, C = 64, 56, 56, 256
N_CORES = 8
B_LOC = B // N_CORES                # 8 batches per core
N_LOC = B_LOC * W * H               # 25088 pixels per core
N_TOT = B * W * H                   # 200704 pixels total
P = 128                             # partitions
UJ = 14                             # pixel-tiles (units) per chunk
CPX = UJ * P                        # 1792 pixels per chunk
NCHUNK = N_LOC // CPX               # 14 chunks per core
EPS = 1e-3
ITER_NUM = 5

assert NCHUNK * CPX == N_LOC

_STATE = {}


def _build_nc(variant=()):
    import concourse.bacc as bacc
    import concourse.tile as tile
    from concourse import mybir
    from contextlib import ExitStack

    f32 = mybir.dt.float32
    f16 = mybir.dt.float16
    Alu = mybir.AluOpType
    Act = mybir.ActivationFunctionType
    Axis = mybir.AxisListType

    nc = bacc.Bacc("TRN2", target_bir_lowering=False, debug=False,
                   num_devices=N_CORES)

    x = nc.dram_tensor("x", [N_LOC, C], f32, kind="ExternalInput").ap()
    y = nc.dram_tensor("y", [N_LOC, C], f32, kind="ExternalOutput").ap()
    c_id16 = nc.dram_tensor("c_id16", [P, P], f16, kind="ExternalInput").ap()
    c_eye = nc.dram_tensor("c_eye", [P, P], f32, kind="ExternalInput").ap()
    c_epseye = nc.dram_tensor("c_epseye", [P, P], f32, kind="ExternalInput").ap()
    c_mask = nc.dram_tensor("c_mask", [P, P], f32, kind="ExternalInput").ap()

    with tile.TileContext(nc) as tc, ExitStack() as octx:
        # ---------------- long-lived pools
        consts = octx.enter_context(tc.tile_pool(name="consts", bufs=1))
        resp = octx.enter_context(tc.tile_pool(name="resident", bufs=1))
        statp = octx.enter_context(tc.tile_pool(name="stats", bufs=1))

        id16 = consts.tile([P, P], f16, name="id16")
        eye = consts.tile([P, P], f32, name="eye")
        epseye = consts.tile([P, P], f32, name="epseye")
        mask = consts.tile([P, P], f32, name="mask")
        nc.gpsimd.dma_start(out=id16, in_=c_id16)
        nc.gpsimd.dma_start(out=eye, in_=c_eye)
        nc.gpsimd.dma_start(out=epseye, in_=c_epseye)
        nc.gpsimd.dma_start(out=mask, in_=c_mask)

        # stats block: [G_a | G_b | sum_a | sum_b] (128, 258) f32
        statsb = statp.tile([P, 2 * P + 2], f32, name="statsb")
        nc.vector.memset(statsb, 0.0)

        # channel-major fp16 resident tiles: one per (chunk, half)
        res = [[resp.tile([P, UJ, P], f16, name=f"res_{c}_{h}")
                for h in range(2)] for c in range(NCHUNK)]

        xv = x.rearrange("(c j p) ch -> c p j ch", p=P, j=UJ)
        yv = y.rearrange("(c j p) ch -> c p j ch", p=P, j=UJ)

        nrep = 1
        for v in variant:
            if v.startswith("rep"):
                nrep = int(v[3:])
        for _rep in range(nrep):
         # ================= PASS 1 =================
         with ExitStack() as ctx:
             loadp = ctx.enter_context(tc.tile_pool(name="loadp", bufs=2))
             castp = ctx.enter_context(tc.tile_pool(name="castp", bufs=2))
             gps = ctx.enter_context(tc.tile_pool(name="gpsum", bufs=1, space="PSUM"))
             trps = ctx.enter_context(tc.tile_pool(name="trpsum", bufs=4, space="PSUM"))

             g_ps = [gps.tile([P, P], f32, name=f"G_{h}") for h in range(2)]

             for ci in range(NCHUNK):
                 xt = loadp.tile([P, UJ, C], f32, name="xt")
                 nc.gpsimd.dma_start(out=xt, in_=xv[ci])
                 xh = castp.tile([P, UJ, C], f16, name="xh")
                 nc.vector.tensor_copy(out=xh, in_=xt)

                 # Gram accumulation (fp16 in, f32 PSUM): G_h += T_h^T @ T_h
                 for j in range(UJ):
                     first = ci == 0 and j == 0
                     last = ci == NCHUNK - 1 and j == UJ - 1
                     if "nogram" in variant:
                         continue
                     for h in range(2):
                         sl = xh[:, j, h * P:(h + 1) * P]
                         nc.tensor.matmul(g_ps[h], sl, sl, start=first,
                                          stop=last, skip_group_check=True)

                 # PE transpose each (128px,128ch) tile -> channel-major fp16
                 for h in range(2 if "notr" not in variant else 0):
                     for b0 in range(0, UJ, 4):
                         bn = min(4, UJ - b0)
                         tp = trps.tile([P, 4, P], f16, name="tp")
                         for k in range(bn):
                             nc.tensor.matmul(
                                 tp[:, k, :], xh[:, b0 + k, h * P:(h + 1) * P],
                                 id16, is_transpose=True, skip_group_check=True)
                         nc.scalar.activation(
                             out=res[ci][h][:, b0:b0 + bn, :], in_=tp[:, :bn, :],
                             func=Act.Copy)

                 # channel sums from the (already rounded) fp16 resident tiles
                 for h in range(2 if "notr" not in variant else 0):
                     csum = castp.tile([P, 1], f32, name="csum")
                     nc.vector.tensor_reduce(out=csum, in_=res[ci][h],
                                             axis=Axis.XY, op=Alu.add)
                     nc.vector.tensor_add(
                         out=statsb[:, 2 * P + h:2 * P + h + 1],
                         in0=statsb[:, 2 * P + h:2 * P + h + 1], in1=csum)

             # move Gram PSUM -> stats block
             for h in range(2 if "nogram" not in variant else 0):
                 nc.scalar.activation(out=statsb[:, h * P:(h + 1) * P],
                                      in_=g_ps[h], func=Act.Copy)

         # ================= ALL-REDUCE =================
         with ExitStack() as ctx:
             dramp = ctx.enter_context(tc.tile_pool(name="dram", bufs=1, space="DRAM"))
             cc_in = dramp.tile([P, 2 * P + 2], f32, name="cc_in")
             cc_out = dramp.tile([P, 2 * P + 2], f32, name="cc_out")
             arst = statp.tile([P, 2 * P + 2], f32, name="arst")
             if "nocc" in variant:
                 nc.vector.tensor_scalar_mul(out=arst, in0=statsb,
                                             scalar1=float(N_CORES))
             else:
                 nc.gpsimd.dma_start(out=cc_in, in_=statsb)
                 nc.gpsimd.collective_compute(
                     "AllReduce", mybir.AluOpType.add,
                     replica_groups=[list(range(N_CORES))],
                     ins=[cc_in.opt()], outs=[cc_out.opt()])
                 nc.gpsimd.dma_start(out=arst, in_=cc_out)

             # ============= Newton-Schulz (per half) =============
             nsp = ctx.enter_context(tc.tile_pool(name="nsp", bufs=6))
             nps = ctx.enter_context(tc.tile_pool(name="nspsum", bufs=4, space="PSUM"))

             wm16 = [statp.tile([P, P], f16, name=f"wm16_{h}") for h in range(2)]
             nmu = [statp.tile([P, 1], f32, name=f"nmu_{h}") for h in range(2)]

             for h in range(2):
                 arG = arst[:, h * P:(h + 1) * P]
                 s_col = arst[:, 2 * P + h:2 * P + h + 1]

                 # -mean column (bias for pass 2)
                 nc.scalar.activation(out=nmu[h], in_=s_col, func=Act.Identity,
                                      scale=-1.0 / N_TOT)

                 # mu as row 0 of a zero (128,128) tile, via PE transpose
                 colpad = nsp.tile([P, P], f32, name="colpad", tag="nsbig")
                 nc.vector.memset(colpad, 0.0)
                 nc.scalar.activation(out=colpad[:, 0:1], in_=s_col,
                                      func=Act.Identity, scale=1.0 / N_TOT)
                 rp_ps = nps.tile([P, P], f32, name="rp_ps", tag="nsps")
                 nc.tensor.matmul(rp_ps, colpad, eye, is_transpose=True,
                                  skip_group_check=True)
                 rowpad = nsp.tile([P, P], f32, name="rowpad", tag="nsbig")
                 nc.scalar.activation(out=rowpad, in_=rp_ps, func=Act.Copy)

                 # outer product mu mu^T (only row 0 of rowpad is nonzero)
                 o_ps = nps.tile([P, P], f32, name="o_ps", tag="nsps")
                 nc.tensor.matmul(o_ps, rowpad, rowpad, skip_group_check=True)
                 osc = nsp.tile([P, P], f32, name="osc", tag="nsbig")
                 nc.scalar.activation(out=osc, in_=o_ps, func=Act.Identity,
                                      scale=-(1.0 - EPS))

                 # sigma = mask * ((1-eps)/N * G - (1-eps) * mu mu^T) + eps*I
                 sig = nsp.tile([P, P], f32, name="sig", tag="sig")
                 nc.vector.scalar_tensor_tensor(
                     out=sig, in0=arG, scalar=(1.0 - EPS) / N_TOT, in1=osc,
                     op0=Alu.mult, op1=Alu.add)
                 nc.vector.tensor_mul(out=sig, in0=sig, in1=mask)
                 nc.vector.tensor_add(out=sig, in0=sig, in1=epseye)

                 # per-group trace, spread back to rows via mask matmul
                 djunk = nsp.tile([P, P], f32, name="djunk", tag="nsbig")
                 dcol = nsp.tile([P, 1], f32, name="dcol", tag="nssmall")
                 nc.vector.tensor_mul(out=djunk, in0=sig, in1=eye)
                 nc.vector.reduce_sum(out=dcol, in_=djunk, axis=Axis.X)
                 tv_ps = nps.tile([P, 1], f32, name="tv_ps", tag="nsps")
                 nc.tensor.matmul(tv_ps, mask, dcol, skip_group_check=True)
                 tvec = nsp.tile([P, 1], f32, name="tvec", tag="nssmall")
                 nc.scalar.activation(out=tvec, in_=tv_ps, func=Act.Copy)
                 rinv = nsp.tile([P, 1], f32, name="rinv", tag="nssmall")
                 nc.vector.reciprocal(out=rinv, in_=tvec)

                 sign = nsp.tile([P, P], f32, name="sign", tag="sign")
                 nc.vector.tensor_scalar_mul(out=sign, in0=sig, scalar1=rinv)

                 # P_{k+1} = 1.5 P - 0.5 P^3 sigma_n ; P_0 = I
                 ps_t = nsp.tile([P, P], f32, name=f"ps_{h}", tag="ps")
                 nc.vector.tensor_copy(out=ps_t, in_=eye)
                 for _ in range(ITER_NUM):
                     p2ps = nps.tile([P, P], f32, name="p2ps", tag="nsps")
                     nc.tensor.matmul(p2ps, ps_t, ps_t, skip_group_check=True)
                     p2s = nsp.tile([P, P], f32, name="p2s", tag="nsbig")
                     nc.scalar.activation(out=p2s, in_=p2ps, func=Act.Copy)
                     p3ps = nps.tile([P, P], f32, name="p3ps", tag="nsps")
                     nc.tensor.matmul(p3ps, p2s, ps_t, skip_group_check=True)
                     p3s = nsp.tile([P, P], f32, name="p3s", tag="nsbig")
                     nc.scalar.activation(out=p3s, in_=p3ps, func=Act.Copy)
                     tps = nps.tile([P, P], f32, name="tps", tag="nsps")
                     nc.tensor.matmul(tps, p3s, sign, skip_group_check=True)
                     ts = nsp.tile([P, P], f32, name="ts", tag="nsbig")
                     nc.scalar.activation(out=ts, in_=tps, func=Act.Identity,
                                          scale=-0.5)
                     pn = nsp.tile([P, P], f32, name=f"ps_{h}", tag="ps")
                     nc.vector.scalar_tensor_tensor(
                         out=pn, in0=ps_t, scalar=1.5, in1=ts,
                         op0=Alu.mult, op1=Alu.add)
                     ps_t = pn

                 # wm = P * rsqrt(trace)  (per-row group trace)
                 sq = nsp.tile([P, 1], f32, name="sq", tag="nssmall")
                 nc.scalar.activation(out=sq, in_=tvec, func=Act.Sqrt)
                 rs = nsp.tile([P, 1], f32, name="rs", tag="nssmall")
                 nc.vector.reciprocal(out=rs, in_=sq)
                 wmf = nsp.tile([P, P], f32, name="wmf", tag="nsbig")
                 nc.vector.tensor_scalar_mul(out=wmf, in0=ps_t, scalar1=rs)
                 nc.vector.tensor_copy(out=wm16[h], in_=wmf)

         # ================= PASS 2 =================
         with ExitStack() as ctx:
             stagep = ctx.enter_context(tc.tile_pool(name="stagep", bufs=2))
             yps = ctx.enter_context(tc.tile_pool(name="ypsum", bufs=4, space="PSUM"))

             for ci in range(NCHUNK if "nop2" not in variant else 0):
                 st = stagep.tile([P, UJ, C], f32, name="st")
                 for h in range(2):
                     # subtract mean in place (per-partition bias, fp16)
                     nc.scalar.activation(out=res[ci][h], in_=res[ci][h],
                                          func=Act.Identity, bias=nmu[h])
                     for b0 in range(0, UJ, 4):
                         bn = min(4, UJ - b0)
                         yp = yps.tile([P, 4, P], f32, name="yp")
                         for k in range(bn):
                             nc.tensor.matmul(yp[:, k, :],
                                              res[ci][h][:, b0 + k, :],
                                              wm16[h], skip_group_check=True)
                         nc.vector.tensor_copy(
                             out=st[:, b0:b0 + bn, h * P:(h + 1) * P],
                             in_=yp[:, :bn, :])
                 nc.gpsimd.dma_start(out=yv[ci], in_=st)

    nc.compile()
    return nc


def _get_nc(variant=()):
    key = ("nc",) + tuple(sorted(variant))
    if key not in _STATE:
        _STATE[key] = _build_nc(variant)
    return _STATE[key]


def _consts():
    g16 = np.eye(P, dtype=np.float16)
    eye = np.eye(P, dtype=np.float32)
    epseye = (EPS * np.eye(P)).astype(np.float32)
    mask = np.zeros((P, P), dtype=np.float32)
    for g in range(P // 16):
        mask[g * 16:(g + 1) * 16, g * 16:(g + 1) * 16] = 1.0
    return {"c_id16": g16, "c_eye": eye, "c_epseye": epseye, "c_mask": mask}


def _run(x, trace=False, variant=()):
    from concourse.bass_utils import run_bass_kernel_spmd

    x = np.ascontiguousarray(x, dtype=np.float32).reshape(B, W * H * C)
    consts = _consts()
    in_maps = []
    for i in range(N_CORES):
        m = {"x": np.ascontiguousarray(
            x[i * B_LOC:(i + 1) * B_LOC].reshape(N_LOC, C))}
        m.update(consts)
        in_maps.append(m)

    nc = _get_nc(variant)
    r = run_bass_kernel_spmd(nc, in_maps, core_ids=list(range(N_CORES)),
                             trace=trace)
    out = np.concatenate([r.results[i]["y"].reshape(B_LOC, W, H, C)
                          for i in range(N_CORES)], axis=0)
    return out, r


def kernel(inputs):
    return _run(inputs, trace=False)[0]


if __name__ == "__main__":
    x = np.random.randn(B, W, H, C).astype(np.float32)
    out, _ = _run(x)
    print(out.shape, out.dtype)

